# revision 31
# baseline (speedup 1.0000x reference)
"""Trainium2 Bass kernel for LongcatFlash MoE experts (expert-parallel, 8 cores).

Problem: T=4096 tokens, H=1024, I=512, 32 routed + 8 zero (identity) experts,
top-4 routing, per-expert capacity 768.

Strategy (expert parallelism, fp8 DoubleRow matmuls):
  - Host: replicate the reference routing (stable sort by expert, capacity
    clip), permute tokens to their expert's core, quantize x / weights to
    fp8e4 (weights pre-scaled by SW=32 to clear the e4m3 subnormal range),
    build per-core packed buffers with tokens on the GEMM free dimension.
  - Slot layout: 4 expert slots per core with tiered widths shared across
    cores (slot j holds the experts ranked [8j, 8j+8) by load; width =
    ceil16 of the tier max — moving-operand segments need 16B alignment
    only). All cores run one SPMD program.
  - Device: per slot run the gated MLP as fp8 DoubleRow matmuls (each MM
    contracts 2 k-tiles = 256 rows):
        gu[o, c]  = sum_h guT[h, o] * xT[h, c]      (PSUM = SW * true)
        sil       = Silu(gu_gate / SW)              (scalar engine)
        mid       = (gu_up * SM/SW) * sil -> fp8    (vector engine, = SM*mid)
        y[h, c]   = sum_i dnT[i, h] * mid[i, c]     (PSUM = SW*SM * true)
    y is copied to SBUF as bf16 still scaled by SW*SM; the descale is folded
    into the host-side router-weight multiply (free).
  - x and slot0's gate_up live in per-h-pair tiles so the first matmul only
    waits on its own pair's DMA, not the whole activation load.
  - Host: gather, scale by router weight / (SW*SM), scatter-add per token,
    add the zero-expert weighted-identity term.
"""

import os

import numpy as np

N_CORES = 8
R = 32  # routed experts
N_SLOTS = 4
CAPACITY = 768
H = 1024
I_DIM = 512
HT = H // 128  # 8 contraction tiles for gate_up
IT = I_DIM // 128  # 4 contraction tiles for down
WMAX = 512

SW = 32.0  # weight pre-scale (both projections)
SM = 8.0  # mid pre-scale for fp8 storage

LAST_RUN = {}  # filled with exec_time_ns etc. for test harness use


def _route(idx, wts, n_tok):
    """Replicates the reference's capacity-buffer routing exactly.

    Returns per-assignment (expert, token, weight, slot, flat_index) for kept
    routed assignments, sorted by expert (stable), plus zero-expert weights.
    """
    K = idx.shape[1]
    A = n_tok * K
    flat_e = idx.reshape(-1).astype(np.int64)
    flat_t = np.repeat(np.arange(n_tok, dtype=np.int64), K)
    flat_w = wts.reshape(-1)
    order = np.argsort(flat_e, kind="stable")
    se = flat_e[order]
    st = flat_t[order]
    sw = flat_w[order]
    counts = np.bincount(flat_e, minlength=R + 8)
    starts = np.cumsum(counts) - counts
    pos = np.arange(A, dtype=np.int64) - starts[se]
    valid = (se < R) & (pos < CAPACITY)
    zero_w = np.where(idx >= R, wts, 0.0).sum(axis=1)
    return (
        se[valid],
        st[valid],
        sw[valid],
        pos[valid],
        order[valid],
        zero_w,
    )


_BUILD_CACHE = {}

def _build_bass(widths):
    import concourse.bacc as bacc
    import concourse.bass as bass
    import concourse.mybir as mybir
    from concourse import tile

    key = tuple(widths)
    if key in _BUILD_CACHE:
        return _BUILD_CACHE[key]

    FT = mybir.dt.float32
    F8 = mybir.dt.float8e4
    BF = mybir.dt.bfloat16
    DR = mybir.MatmulPerfMode.DoubleRow
    silu_fn = mybir.ActivationFunctionType.Silu

    WTOT = sum(widths)
    xoffs = [sum(widths[:s]) for s in range(N_SLOTS)]

    W0 = widths[0]
    nc = bacc.Bacc(None)
    # slot0: gu pair and x pair fused per h-pair -> one DMA covers both
    c0_d = nc.declare_dram_parameter("c0", [4, 128, 2, 1024 + W0], F8, isOutput=False)
    # x pair-major: [pair, 128, h01, WTOT] (slot0 range unused)
    xt_d = nc.declare_dram_parameter("xt", [4, 128, 2, WTOT], F8, isOutput=False)
    gu_d = nc.declare_dram_parameter("guw", [N_SLOTS, 128, HT, 1024], F8, isOutput=False)
    dn_d = nc.declare_dram_parameter("dnw", [N_SLOTS, 128, IT, 1024], F8, isOutput=False)
    yt_d = nc.declare_dram_parameter("yt", [128, HT * WTOT], BF, isOutput=True)

    # A-phase oi order: last-emitted STT feeds the D-phase group emitted last
    A_ORDER = (2, 3, 0, 1)

    with tile.TileContext(nc) as tc:
        with (
            tc.tile_pool(name="xpool", bufs=4) as xpool,
            tc.tile_pool(name="gu0pool", bufs=4) as gu0pool,
            tc.tile_pool(name="gupool", bufs=N_SLOTS - 1) as gupool,
            tc.tile_pool(name="dnpool", bufs=N_SLOTS) as dnpool,
            tc.tile_pool(name="midpool", bufs=2) as midpool,
            tc.tile_pool(name="silpool", bufs=8) as silpool,
            tc.tile_pool(name="ypool", bufs=2) as ypool,
            tc.tile_pool(name="pgpool", bufs=2, space="PSUM") as pgpool,
            tc.tile_pool(name="pupool", bufs=2, space="PSUM") as pupool,
            tc.tile_pool(name="pypool", bufs=3, space="PSUM") as pypool,
        ):
            # ---- DMA in: everything up front, slot-0 pair 0 first ----
            xps = [
                [None]
                + [
                    xpool.tile([128, 2, widths[s]], F8, tag=f"xp{q}_{s}", name=f"xp{q}_{s}", bufs=1)
                    for s in range(1, N_SLOTS)
                ]
                for q in range(4)
            ]
            gu0s = [
                gu0pool.tile([128, 2, 1024 + W0], F8, tag="gu0", name=f"gu0p{q}")
                for q in range(4)
            ]
            dnts = [dnpool.tile([128, IT, 1024], F8, tag="dn", name=f"dnt{s}") for s in range(N_SLOTS)]
            guts = [None] + [
                gupool.tile([128, HT, 1024], F8, tag="gu", name=f"gut{s}") for s in range(1, N_SLOTS)
            ]
            # ALL input DMAs serialize on the sync queue in exact need-order:
            # early HBM bandwidth goes to slot 0 only, so the PE is never
            # starved by transfers it does not yet need.
            # slot0 fused pairs: 0,2 on sync and 1,3 on gpsimd so two trigger
            # streams run in parallel and gpsimd carries nothing else early.
            # Everything else serializes on sync in exact need-order.
            for q in range(4):
                eng = nc.sync if q % 2 == 0 else nc.gpsimd
                eng.dma_start(gu0s[q][:], c0_d[q])
            for s in range(1, N_SLOTS):
                x0 = xoffs[s]
                nc.sync.dma_start(guts[s][:], gu_d[s])
                for q in range(4):
                    nc.sync.dma_start(xps[q][s][:], xt_d[q][:, :, x0 : x0 + widths[s]])
                if s == 1:
                    # dn0 is not needed until D0 (~21us); keep it off the
                    # critical gu1/x-s1 stretch that gates A1 (~17us)
                    nc.sync.dma_start(dnts[0][:], dn_d[0])
                nc.sync.dma_start(dnts[s][:], dn_d[s])

            # ---- compute: PE order A0 A1 D0 A2 D1 A3 D2 D3 ----
            # D(s) trails A(s) by a full A phase, so its mids are long done
            # and the PE never stalls waiting for silu/mult at phase entry.
            mids = {}

            def gen_A(s):
                W = widths[s]

                def gu_slice(j, c0, c1, _s=s):
                    if _s == 0:
                        return gu0s[j][:, :, c0:c1]
                    return guts[_s][:, 2 * j : 2 * j + 2, c0:c1]

                def x_slice(j, _s=s):
                    if _s == 0:
                        return gu0s[j][:, :, 1024 : 1024 + W0]
                    return xps[j][_s][:]

                # A phase: gate/up projections -> sil (scalar) -> mid (vector).
                # Pair-outer waves: within a wave the contraction pair is the
                # outer loop, so pair j's data is not needed until 4 MMs in.
                mid = midpool.tile([128, IT, WMAX], F8, tag="mid")
                mids[s] = mid
                for wave in (A_ORDER[:2], A_ORDER[2:]):
                    ps = {}
                    for oi in wave:
                        ps[("g", oi)] = pgpool.tile([128, WMAX], FT, tag="pg", name=f"pg{oi}")
                        ps[("u", oi)] = pupool.tile([128, WMAX], FT, tag="pu", name=f"pu{oi}")
                    for j in range(4):
                        for oi in wave:
                            nc.tensor.matmul(
                                ps[("g", oi)][:, :W],
                                gu_slice(j, oi * 128, (oi + 1) * 128),
                                x_slice(j),
                                start=(j == 0),
                                stop=(j == 3),
                                perf_mode=DR,
                            )
                            nc.tensor.matmul(
                                ps[("u", oi)][:, :W],
                                gu_slice(j, 512 + oi * 128, 512 + (oi + 1) * 128),
                                x_slice(j),
                                start=(j == 0),
                                stop=(j == 3),
                                perf_mode=DR,
                            )
                    for oi in wave:
                        sil = silpool.tile([128, WMAX], FT, tag="sil")
                        nc.scalar.activation(
                            sil[:, :W], ps[("g", oi)][:, :W], silu_fn, scale=1.0 / SW
                        )
                        nc.vector.scalar_tensor_tensor(
                            mid[:, oi, :W], ps[("u", oi)][:, :W], SM / SW, sil[:, :W],
                            mybir.AluOpType.mult, mybir.AluOpType.mult,
                        )

            def gen_D(s):
                W = widths[s]
                dnt = dnts[s]
                mid = mids.pop(s)
                # D phase: down projection, per-h PSUM groups and copies
                yoff = HT * xoffs[s]
                ywide = ypool.tile([128, HT * W], BF, tag=f"yo{s}", bufs=1, name=f"yw{s}")
                for h in range(HT):
                    py = pypool.tile([128, WMAX], FT, tag="py")
                    # j=1 first: its mids (oi 2,3) are produced first by A_ORDER
                    for j in (1, 0):
                        nc.tensor.matmul(
                            py[:, :W],
                            dnt[:, 2 * j : 2 * j + 2, h * 128 : (h + 1) * 128],
                            mid[:, 2 * j : 2 * j + 2, :W],
                            start=(j == 1),
                            stop=(j == 0),
                            perf_mode=DR,
                        )
                    dst = ywide[:, h * W : (h + 1) * W]
                    if h % 2 == 0:
                        nc.vector.tensor_copy(dst, py[:, :W])
                    else:
                        nc.scalar.copy(dst, py[:, :W])
                    if s < N_SLOTS - 1:
                        if h == 3:
                            nc.gpsimd.dma_start(
                                yt_d[:, yoff : yoff + 4 * W], ywide[:, : 4 * W]
                            )
                        elif h == 7:
                            nc.gpsimd.dma_start(
                                yt_d[:, yoff + 4 * W : yoff + 8 * W], ywide[:, 4 * W :]
                            )
                    elif h in (1, 3, 5):
                        # last slot: pair DMAs spread over idle queues so the
                        # final triggers fire in parallel
                        eng = {1: nc.gpsimd, 3: nc.gpsimd, 5: nc.scalar}[h]
                        eng.dma_start(
                            yt_d[:, yoff + (h - 1) * W : yoff + (h + 1) * W],
                            ywide[:, (h - 1) * W : (h + 1) * W],
                        )
                    elif h == 6:
                        nc.gpsimd.dma_start(
                            yt_d[:, yoff + 6 * W : yoff + 7 * W], ywide[:, 6 * W : 7 * W]
                        )
                    elif h == 7:
                        # the very last transfer is a single stripe -> short drain
                        nc.sync.dma_start(
                            yt_d[:, yoff + 7 * W : yoff + 8 * W], ywide[:, 7 * W :]
                        )

            gen_A(0)
            gen_A(1)
            gen_D(0)
            gen_A(2)
            gen_D(1)
            gen_A(3)
            gen_D(2)
            gen_D(3)

    nc.finalize()
    _BUILD_CACHE[key] = nc
    return nc


def _install_trace_shims():
    """Make trace=True usable in this image: provide the NTFF hook module and
    neutralize the artifact upload (no bucket access needed for local use)."""
    import sys
    import types

    try:
        import antenv.axon_hooks  # noqa: F401
    except ImportError:
        hook = None
        try:
            from trn_agent_boot.trn_boot import _ntff_profile_via_ctypes

            hook = _ntff_profile_via_ctypes("/opt/axon/libaxon_pjrt.so")
        except Exception:
            hook = None
        mod = types.ModuleType("antenv.axon_hooks")
        mod._hook = hook
        mod.get_axon_ntff_profile_hook = lambda: mod._hook
        mod.set_axon_ntff_profile_hook = lambda h: setattr(mod, "_hook", h)
        sys.modules["antenv.axon_hooks"] = mod

    import concourse.bass_utils as bu

    orig_upload = bu.upload_artifacts

    def safe_upload(tmpdir):
        try:
            return orig_upload(tmpdir)
        except Exception:
            return tmpdir
    bu.upload_artifacts = safe_upload


def kernel(**inputs):
    import ml_dtypes

    from concourse.bass_utils import run_bass_kernel_spmd

    F8NP = ml_dtypes.float8_e4m3

    hidden = np.ascontiguousarray(np.asarray(inputs["hidden_states"], dtype=np.float32))
    idx = np.asarray(inputs["top_k_index"]).astype(np.int64)
    wts = np.asarray(inputs["top_k_weights"], dtype=np.float32)
    gup = np.asarray(inputs["gate_up_proj"], dtype=np.float32)
    dnp = np.asarray(inputs["down_proj"], dtype=np.float32)

    n_tok = hidden.shape[0]
    K = idx.shape[1]

    ve, vt, vw, vp, va, zero_w = _route(idx, wts, n_tok)
    cnts = np.bincount(ve, minlength=R)
    estarts = np.cumsum(cnts) - cnts

    # tiered slot widths: slot j holds experts ranked [8j, 8j+8) by load
    rank = np.argsort(-cnts, kind="stable")
    widths = []
    for j in range(N_SLOTS):
        w = int(((cnts[rank[8 * j]] + 15) // 16) * 16)
        widths.append(max(64, min(WMAX, w)))
    assert cnts.max() <= WMAX, "expert load exceeds 512; unsupported"
    WTOT = sum(widths)
    xoffs = [sum(widths[:s]) for s in range(N_SLOTS)]
    # slot_expert[c][s] = global expert id
    slot_expert = [[int(rank[8 * s + c]) for s in range(N_SLOTS)] for c in range(N_CORES)]

    xq = hidden.astype(F8NP)  # [T, H] quantized once

    in_maps = []
    for c in range(N_CORES):
        xt = np.zeros((HT, 128, WTOT), dtype=F8NP)
        guw = np.empty((N_SLOTS, 128, HT, 1024), dtype=F8NP)
        dnw = np.empty((N_SLOTS, 128, IT, 1024), dtype=F8NP)
        for s in range(N_SLOTS):
            ge = slot_expert[c][s]
            s0, cnt = estarts[ge], cnts[ge]
            if cnt:
                toks = vt[s0 : s0 + cnt]
                # [cnt, H] -> [H, cnt] -> [HT, 128, cnt]
                xb = xq[toks].T.reshape(HT, 128, cnt)
                xt[:, :, xoffs[s] : xoffs[s] + cnt] = xb
            # guT[h, m] tiles: [128p, HT, 1024m]
            gw = (gup[ge].T.reshape(HT, 128, 1024) * SW).astype(F8NP).transpose(1, 0, 2)
            # dnT[i, h] tiles: [128p, IT, 1024h]
            dw = (dnp[ge].T.reshape(IT, 128, 1024) * SW).astype(F8NP).transpose(1, 0, 2)
            guw[s] = gw
            dnw[s] = dw
        # pair-major x: [pair, 128, h01, WTOT]
        xt = xt.reshape(4, 2, 128, WTOT).transpose(0, 2, 1, 3)
        # slot0 fused gu+x pairs: [pair, 128, 2, 1024+W0]
        W0 = widths[0]
        c0 = np.empty((4, 128, 2, 1024 + W0), dtype=F8NP)
        c0[:, :, :, :1024] = guw[0].reshape(128, 4, 2, 1024).transpose(1, 0, 2, 3)
        c0[:, :, :, 1024:] = xt[:, :, :, xoffs[0] : xoffs[0] + W0]
        in_maps.append({
            "c0": np.ascontiguousarray(c0),
            "xt": np.ascontiguousarray(xt),
            "guw": np.ascontiguousarray(guw),
            "dnw": np.ascontiguousarray(dnw),
        })

    nc = _build_bass(widths)

    trace = bool(int(os.environ.get("KERNEL_TRACE", "0")))
    if trace:
        _install_trace_shims()
    res = run_bass_kernel_spmd(nc, in_maps, list(range(N_CORES)), trace=trace)
    LAST_RUN["exec_time_ns"] = res.exec_time_ns
    LAST_RUN["mean_exec_time_ns"] = res.mean_exec_time_ns
    LAST_RUN["instructions_and_trace"] = res.instructions_and_trace
    LAST_RUN["profile_json"] = res.profile_json

    # ---- combine on host (descale by SW*SM folded into router weight) ----
    out = hidden * zero_w[:, None].astype(np.float32)
    acc = np.zeros((n_tok * K, H), dtype=np.float32)
    descale = 1.0 / (SW * SM)
    for c in range(N_CORES):
        yt = np.asarray(res.results[c]["yt"]).astype(np.float32)  # [128, HT*WTOT]
        for s in range(N_SLOTS):
            ge = slot_expert[c][s]
            s0, cnt = estarts[ge], cnts[ge]
            if cnt == 0:
                continue
            W = widths[s]
            ys = yt[:, HT * xoffs[s] : HT * (xoffs[s] + W)].reshape(128, HT, W)
            y = ys.transpose(1, 0, 2).reshape(H, W)[:, :cnt].T  # [cnt, H]
            acc[va[s0 : s0 + cnt]] = y * (vw[s0 : s0 + cnt, None] * descale)
    out += acc.reshape(n_tok, K, H).sum(axis=1)
    return out
